# revision 48
# baseline (speedup 1.0000x reference)
"""Trainium2 Bass kernel for nn_DRL4SSP (pointer-network greedy decode).

Strategy: pure data-parallel over batch B=64 across 8 NeuronCores (8 items
per core, 2 pipeline groups of 4). The 127 sequential decode steps run fully
on-chip; the per-step recurrence is latency-bound, so the design minimizes
the serial chain:

  * argmax tail: logits -> gpsimd partition_all_reduce(max) -> is_equal
    one-hot, all in the native [s, b] layout (no PE transposes, no
    Max/MaxIndex on the hot path).
  * GRU input gates: W_ih@W_dec folded on the host; per-item GI^T = static^T
    @ (W_ih_g W_dec)^T precomputed in the prologue, so the gates for step
    t+1 are one-hot gather matmuls that accumulate onto W_hh@h PSUM
    preloads issued off the critical path during step t.
  * tour_idx / tour_logp bookkeeping is reconstructed post-loop from the
    stored logits (Max/MaxIndex + exp/ln per 128-column chunk).
  * softmax normalization deferred: U2 = W2SH@exp(attn1), scaled by the
    partition-replicated reciprocal sum afterwards (reciprocal runs in
    parallel with the U2 matvecs).
  * heterogeneous pipeline groups: group 0 runs stage-2 as a [128,512]
    DVE broadcast-add + one big ACT tanh, group 1 runs it as per-item
    ACT tanh with the u2 column as SBUF bias (no DVE add) - the two
    groups then use complementary engines and contend less.

All argmax-affecting arithmetic is fp32 (bf16/f32r measured to flip tours).
"""
import sys
import numpy as np

for _p in ("/opt/trn_rl_repo",):
    if _p not in sys.path:
        sys.path.insert(0, _p)

B, SS, DS, H, S = 64, 8, 4, 128, 128
NCORES = 8
BL = B // NCORES          # batch items per core = 8
NG = 2                    # pipeline groups per core
GB = BL // NG             # batch items per group = 4
NSTEP = S - 1             # 127
NEG = -1e30


def _build_nc(n_steps=NSTEP, bench_loop=1):
    from contextlib import ExitStack, nullcontext
    import concourse.bass as bass
    import concourse.tile as tile
    from concourse import bacc, mybir, bass_isa

    f32 = mybir.dt.float32
    u32 = mybir.dt.uint32
    AF = mybir.ActivationFunctionType
    OP = mybir.AluOpType

    nc = bacc.Bacc("TRN2", target_bir_lowering=False, debug=False,
                   enable_asserts=False)

    # ---- DRAM I/O ----
    din = {}
    def dram_in(name, shape):
        din[name] = nc.dram_tensor(name, shape, f32, kind="ExternalInput").ap()
    # weight constants in two packed tensors: the narrow [SS, .] blocks the
    # prologue needs first (small DMA), and the [128, .] loop weights that
    # are only read ~20us later (DMA'd last, overlaps the prologue)
    dram_in("Wpack8", [SS + DS, 6 * H])
    dram_in("sd12", [SS + DS, BL * S])     # [static;dynamic] stacked, [i,(b,s)]
    dram_in("penT0", [S, BL])              # penalty, transposed [s, b]
    dram_in("Wpack128", [H, 5 * H + 2])
    nchunk = (GB * n_steps + S - 1) // S           # logit chunks per group
    out_idx = nc.dram_tensor("out_idx2", [S, NG * nchunk], u32,
                             kind="ExternalOutput").ap()
    out_logp = nc.dram_tensor("out_logp2", [S, NG * nchunk], f32,
                              kind="ExternalOutput").ap()

    with ExitStack() as ctx:
        tc = ctx.enter_context(tile.TileContext(nc))
        cpool = ctx.enter_context(tc.tile_pool(name="consts", bufs=1))
        state = ctx.enter_context(tc.tile_pool(name="state", bufs=1))
        work = ctx.enter_context(tc.tile_pool(name="work", bufs=2))

        if bench_loop > 1:
            loop_cm = tc.For_i(0, bench_loop, 1)
        else:
            loop_cm = None
        with (loop_cm if loop_cm is not None else nullcontext()):
            # ---- load constants to SBUF (matmuls read the DMA'd tiles
            # directly; no staging copies) ----
            cs = {}
            for nm, ap in din.items():
                raw = cpool.tile(list(ap.shape), f32, tag=f"r_{nm}", name=f"r_{nm}")
                nc.sync.dma_start(raw[:], ap[:])
                cs[nm] = raw
            wp8 = cs.pop("Wpack8")
            wp = cs.pop("Wpack128")
            off = [0]
            def w8slice(p=SS):
                o = off[0]; off[0] += H
                return wp8[0:p, o:o + H]
            for nm in ("WB1T", "WB2T"):
                cs[nm] = w8slice(p=SS + DS)
            for nm in ("WC2T", "WCrT", "WCzT", "WCnT"):
                cs[nm] = w8slice()
            cs["staticT8"] = cs["sd12"][0:SS, :]
            off = [0]
            def wslice(w):
                o = off[0]; off[0] += w
                return wp[:, o:o + w]
            cs["w1hT"] = wslice(H)
            cs["WhhT_r"] = wslice(H)
            cs["WhhT_z"] = wslice(H)
            cs["WhhT_nh"] = wslice(H)
            cs["I128"] = wslice(H)
            cs["vv1c"] = wslice(1)
            cs["vv2c"] = wslice(1)

            # ---- persistent state ----
            base1P = state.tile([H, BL * S], f32, tag="base1P")
            base2P = state.tile([H, BL * S], f32, tag="base2P")
            W2SHT = state.tile([S, BL * H], f32, tag="W2SHT")
            GIrT = state.tile([S, BL * H], f32, tag="GIrT")
            GIzT = state.tile([S, BL * H], f32, tag="GIzT")
            GInT = state.tile([S, BL * H], f32, tag="GInT")
            hT = [state.tile([H, GB], f32, tag=f"hT{g}", name=f"hT_{g}")
                  for g in range(NG)]
            ohT = [state.tile([S, GB], f32, tag=f"ohT{g}", name=f"ohT_{g}")
                   for g in range(NG)]
            mxT = [state.tile([S, GB], f32, tag=f"mxT{g}", name=f"mxT_{g}")
                   for g in range(NG)]
            penaltyT = [state.tile([S, GB], f32, tag=f"penT{g}", name=f"penT_{g}")
                        for g in range(NG)]
            logbT = [state.tile([S, GB * n_steps], f32, tag=f"logbT{g}",
                                name=f"logbT_{g}") for g in range(NG)]

            for g in range(NG):
                nc.vector.memset(hT[g][:], 0.0)
                nc.vector.memset(ohT[g][:], 0.0)
                nc.vector.tensor_copy(out=penaltyT[g][:],
                                      in_=cs["penT0"][:, g * GB:(g + 1) * GB])

            # ---- prologue: bases + per-item folded/transposed weights ----
            # (pools opened via ctx so the decode loop is NOT barriered by a
            # scope exit; banks: pro 2+2 + loop 4 = 8)
            ppb = ctx.enter_context(
                tc.tile_pool(name="pro_big", bufs=1, space="PSUM"))
            ppm = ctx.enter_context(
                tc.tile_pool(name="pro_sm", bufs=2, space="PSUM"))
            if True:
                # per-item transposed mats: X_b^T @ WCT  (K = SS = 8);
                # two matmuls share one PSUM tile, drained by one copy
                # (gpsimd cannot read PSUM, so rotate DVE/ACT only)
                ei = 0
                for dst, wc in ((W2SHT, "WC2T"), (GIrT, "WCrT"),
                                (GIzT, "WCzT"), (GInT, "WCnT")):
                    for b in range(0, BL, 2):
                        pt = ppm.tile([S, 2 * H], f32, tag="pros")
                        for j in range(2):
                            ssl = slice((b + j) * S, (b + j + 1) * S)
                            nc.tensor.matmul(pt[:, j * H:(j + 1) * H],
                                             cs["staticT8"][:, ssl], cs[wc][:],
                                             start=True, stop=True)
                        hsl = slice(b * H, (b + 2) * H)
                        if ei % 2:
                            nc.scalar.copy(dst[:, hsl], pt[:])
                        else:
                            nc.vector.tensor_copy(out=dst[:, hsl], in_=pt[:])
                        ei += 1
                # base = [WBs|WBd] @ [static;dyn]: one K=12 matmul per half
                for dst, wb in ((base1P, "WB1T"), (base2P, "WB2T")):
                    for half in range(2):
                        sl = slice(half * 512, half * 512 + 512)
                        pt = ppb.tile([H, 512], f32, tag="pro")
                        nc.tensor.matmul(pt[:], cs[wb][:], cs["sd12"][:, sl],
                                         start=True, stop=True)
                        nc.vector.tensor_copy(out=dst[:, sl], in_=pt[:])

            # ---- main-loop PSUM pools (per group) ----
            psA = [ctx.enter_context(
                tc.tile_pool(name=f"Ag{g}", bufs=1, space="PSUM")) for g in range(NG)]
            psB = [ctx.enter_context(
                tc.tile_pool(name=f"Bg{g}", bufs=1, space="PSUM")) for g in range(NG)]
            # gates: R 0:4 | Z 4:8 | N 8:12 | H2 12:16, then U1 16:20
            gA = [psA[g].tile([H, 32], f32, tag="gA", name=f"gA_{g}") for g in range(NG)]
            # A1T 0:4 | S1 4:8 | U2 8:12 | A2T 12:16
            gB = [psB[g].tile([H, 32], f32, tag="gB", name=f"gB_{g}") for g in range(NG)]

            def preload(g):
                # W_hh parts of next step's gates (h already updated)
                R, Z = gA[g][:, 0:4], gA[g][:, 4:8]
                H2 = gA[g][:, 12:16]
                nc.tensor.matmul(R, cs["WhhT_r"], hT[g][:],
                                 start=True, stop=False, skip_group_check=True)
                nc.tensor.matmul(Z, cs["WhhT_z"], hT[g][:],
                                 start=True, stop=False, skip_group_check=True)
                nc.tensor.matmul(H2, cs["WhhT_nh"], hT[g][:],
                                 start=True, stop=True)

            for g in range(NG):
                preload(g)

            AFt, AFe = AF.Tanh, AF.Exp

            def step(t, g):
                gs = slice(g * GB * S, (g + 1) * GB * S)  # group (b,s) cols
                ga, gb_ = gA[g], gB[g]
                R, Z, N, H2 = ga[:, 0:4], ga[:, 4:8], ga[:, 8:12], ga[:, 12:16]
                RZ, U1 = ga[:, 0:8], ga[:, 16:20]
                A1T, S1, U2, A2T = (gb_[:, 0:4], gb_[:, 4:8],
                                    gb_[:, 8:12], gb_[:, 12:16])
                h_g, oh_g = hT[g][:], ohT[g][:]

                # ---- GRU gates: one-hot gathers accumulate onto preloads ----
                for bl in range(GB):
                    b = g * GB + bl
                    hsl = slice(b * H, (b + 1) * H)
                    osl = oh_g[:, bl:bl + 1]
                    nc.tensor.matmul(R[:, bl:bl + 1], GIrT[:, hsl], osl,
                                     start=False, stop=True, skip_group_check=True)
                    nc.tensor.matmul(Z[:, bl:bl + 1], GIzT[:, hsl], osl,
                                     start=False, stop=True, skip_group_check=True)
                    nc.tensor.matmul(N[:, bl:bl + 1], GInT[:, hsl], osl,
                                     start=True, stop=True, skip_group_check=True)

                # ---- GRU elementwise ----
                # critical chain: trz -> q -> nin -> tn -> m -> U1 matmuls;
                # z2/z2c/d1 and the final h update run off the chain.
                trz = work.tile([H, 2 * GB], f32, tag=f"trz{g}")
                nc.scalar.activation(trz[:], RZ, AFt, scale=0.5)
                q = work.tile([H, GB], f32, tag=f"q{g}")
                nc.vector.scalar_tensor_tensor(out=q[:], in0=trz[:, 0:GB],
                                               scalar=1.0, in1=H2,
                                               op0=OP.add, op1=OP.mult)
                nin = work.tile([H, GB], f32, tag=f"nin{g}")
                nc.vector.tensor_tensor(out=nin[:], in0=q[:], in1=N, op=OP.add)
                z2 = work.tile([H, GB], f32, tag=f"z2{g}")      # z
                nc.vector.tensor_scalar(out=z2[:], in0=trz[:, GB:2 * GB],
                                        scalar1=0.5, scalar2=0.5,
                                        op0=OP.mult, op1=OP.add)
                z2c = work.tile([H, GB], f32, tag=f"z2c{g}")    # 1 - z
                nc.vector.tensor_scalar(out=z2c[:], in0=trz[:, GB:2 * GB],
                                        scalar1=-0.5, scalar2=0.5,
                                        op0=OP.mult, op1=OP.add)
                d1 = work.tile([H, GB], f32, tag=f"d1{g}")      # z * h_old
                nc.vector.tensor_tensor(out=d1[:], in0=z2[:], in1=h_g,
                                        op=OP.mult)
                # U1 = w1h @ (m + d1) as two accumulating matmuls so the h
                # update itself is off the critical chain; the d1 half is
                # issued early (d1 is ready before tn).
                nc.tensor.matmul(U1, cs["w1hT"], d1[:], start=True, stop=False)
                tn = work.tile([H, GB], f32, tag=f"tn{g}")
                nc.scalar.activation(tn[:], nin[:], AFt)
                m_ = work.tile([H, GB], f32, tag=f"m{g}")       # (1-z) * n
                nc.vector.tensor_tensor(out=m_[:], in0=z2c[:], in1=tn[:],
                                        op=OP.mult)
                nc.tensor.matmul(U1, cs["w1hT"], m_[:], start=False, stop=True)
                nc.vector.tensor_tensor(out=h_g, in0=m_[:], in1=d1[:], op=OP.add)
                if t < n_steps - 1:
                    preload(g)

                # ---- stage 1 ----
                t1p = work.tile([H, GB * S], f32, tag=f"t1p{g}")
                nc.vector.tensor_tensor(
                    out=t1p[:].rearrange("p (b s) -> p b s", b=GB),
                    in0=base1P[:, gs].rearrange("p (b s) -> p b s", b=GB),
                    in1=U1[:, :, None].broadcast_to((H, GB, S)), op=OP.add)
                t1S = work.tile([H, GB * S], f32, tag=f"t1S{g}")
                nc.scalar.activation(t1S[:], t1p[:], AFt)
                for bl in range(GB):
                    nc.tensor.matmul(A1T[:, bl:bl + 1],
                                     t1S[:, bl * S:(bl + 1) * S], cs["vv1c"],
                                     start=True, stop=True)
                e1T = work.tile([S, GB], f32, tag=f"e1T{g}")
                nc.scalar.activation(e1T[:], A1T, AFe)   # softmax1 w/o max-sub
                s1r = work.tile([S, GB], f32, tag=f"s1r{g}")
                nc.gpsimd.partition_all_reduce(s1r[:], e1T[:], channels=S,
                                               reduce_op=bass_isa.ReduceOp.add)

                # ---- stage 2 (deferred softmax normalization) ----
                for bl in range(GB):
                    b = g * GB + bl
                    nc.tensor.matmul(U2[:, bl:bl + 1],
                                     W2SHT[:, b * H:(b + 1) * H],
                                     e1T[:, bl:bl + 1], start=True, stop=True)
                r1 = work.tile([S, GB], f32, tag=f"r1{g}")
                nc.vector.reciprocal(r1[:], s1r[:])
                u2S = work.tile([H, GB], f32, tag=f"u2S{g}")
                nc.vector.tensor_tensor(out=u2S[:], in0=U2, in1=r1[:],
                                        op=OP.mult)
                t2S = work.tile([H, GB * S], f32, tag=f"t2S{g}")
                if g == 0:
                    t2p = work.tile([H, GB * S], f32, tag=f"t2p{g}")
                    nc.vector.tensor_tensor(
                        out=t2p[:].rearrange("p (b s) -> p b s", b=GB),
                        in0=base2P[:, gs].rearrange("p (b s) -> p b s", b=GB),
                        in1=u2S[:, :, None].broadcast_to((H, GB, S)), op=OP.add)
                    nc.scalar.activation(t2S[:], t2p[:], AFt)
                    for bl in range(GB):
                        nc.tensor.matmul(A2T[:, bl:bl + 1],
                                         t2S[:, bl * S:(bl + 1) * S], cs["vv2c"],
                                         start=True, stop=True)
                else:
                    # ACT-bias path: per-item tanh(base2 + u2), attn matvec
                    # overlaps the next item's tanh; frees DVE for group 0
                    for bl in range(GB):
                        bs_ = slice((g * GB + bl) * S, (g * GB + bl + 1) * S)
                        ls_ = slice(bl * S, (bl + 1) * S)
                        nc.scalar.activation(t2S[:, ls_], base2P[:, bs_], AFt,
                                             bias=u2S[:, bl:bl + 1])
                        nc.tensor.matmul(A2T[:, bl:bl + 1],
                                         t2S[:, ls_], cs["vv2c"],
                                         start=True, stop=True)

                # ---- logits, one-hot via partition all-reduce max ----
                lslot = logbT[g][:, t * GB:(t + 1) * GB]
                nc.vector.tensor_tensor(out=lslot, in0=A2T,
                                        in1=penaltyT[g][:], op=OP.add)
                nc.gpsimd.partition_all_reduce(mxT[g][:], lslot, channels=S,
                                               reduce_op=bass_isa.ReduceOp.max)
                nc.vector.tensor_tensor(out=oh_g, in0=lslot, in1=mxT[g][:],
                                        op=OP.is_equal)
                tsp = work.tile([S, GB], f32, tag=f"tsp{g}")
                nc.gpsimd.tensor_scalar(out=tsp[:], in0=oh_g, scalar1=NEG,
                                        scalar2=None, op0=OP.mult)
                nc.gpsimd.tensor_tensor(out=penaltyT[g][:], in0=penaltyT[g][:],
                                        in1=tsp[:], op=OP.add)

            for t in range(n_steps):
                for g in range(NG):
                    step(t, g)

            # ---- post-loop: ptr = argmax(logits); logp = -ln(sum(exp(l-max)))
            # logbT[g] is [s, (t,b)]; transpose 128-col chunks to [(t,b), s].
            sums = [state.tile([S, nchunk], f32, tag=f"sums{g}",
                               name=f"sums_{g}") for g in range(NG)]
            idxs = [state.tile([S, nchunk], u32, tag=f"idxs{g}",
                               name=f"idxs_{g}") for g in range(NG)]
            eps = ppb   # reuse the (still-open) prologue psum pool
            if True:
                for g in range(NG):
                    nc.vector.memset(sums[g][:], 1.0)
                    for c in range(nchunk):
                        w0 = c * S
                        wid = min(S, GB * n_steps - w0)
                        pt = eps.tile([S, S], f32, tag="epi", name=f"pT{g}{c}")
                        nc.tensor.transpose(pt[0:wid, :],
                                            logbT[g][:, w0:w0 + wid], cs["I128"])
                        # Max/MaxIndex/exp read the transposed PSUM tile
                        # directly; no SBUF staging copy
                        M8 = work.tile([S, 8], f32, tag=f"m8{g}")
                        nc.vector.max(M8[0:wid, :], pt[0:wid, :])
                        I8u = work.tile([S, 8], u32, tag=f"i8{g}")
                        nc.vector.max_index(I8u[0:wid, :], M8[0:wid, :],
                                            pt[0:wid, :])
                        nc.gpsimd.tensor_copy(out=idxs[g][0:wid, c:c + 1],
                                              in_=I8u[0:wid, 0:1])
                        nmx = work.tile([S, 1], f32, tag=f"nm{g}")
                        nc.vector.tensor_scalar(out=nmx[0:wid, :],
                                                in0=M8[0:wid, 0:1],
                                                scalar1=-1.0, scalar2=None,
                                                op0=OP.mult)
                        eb = work.tile([S, S], f32, tag=f"eb{g}")
                        nc.scalar.activation(eb[0:wid, :], pt[0:wid, :], AFe,
                                             bias=nmx[0:wid, :],
                                             accum_out=sums[g][0:wid, c:c + 1])
            # ship raw softmax sums; logp = -log(sum) is applied on the
            # host during unpack (keeps Ln and its ACT table load off-device)
            for g in range(NG):
                nc.sync.dma_start(out_idx[:, g * nchunk:(g + 1) * nchunk],
                                  idxs[g][:])
                nc.sync.dma_start(out_logp[:, g * nchunk:(g + 1) * nchunk],
                                  sums[g][:])

    nc.compile()
    return nc


def host_inputs(static, dynamic, W_s, W_d, W_dec, vv1, ww1, vv2, ww2,
                W_ih, W_hh):
    """Per-core in_maps (layout transforms + tiny weight folds only)."""
    f = np.float32
    ca = np.ascontiguousarray
    blocks8 = [
        np.concatenate([(ww1[:, :H] @ W_s).T,
                        (ww1[:, H:2 * H] @ W_d).T]),   # WB1T   [SS+DS, H]
        np.concatenate([(ww2[:, :H] @ W_s).T,
                        (ww2[:, 2 * H:] @ W_d).T]),    # WB2T   [SS+DS, H]
        (ww2[:, H:2 * H] @ W_s).T,        # WC2T          [SS, H]
        (W_ih[:H] @ W_dec).T,             # WCrT          [SS, H]
        (W_ih[H:2 * H] @ W_dec).T,        # WCzT          [SS, H]
        (W_ih[2 * H:] @ W_dec).T,         # WCnT          [SS, H]
    ]
    wpack8 = np.zeros((SS + DS, 6 * H), f)
    for i, b in enumerate(blocks8):
        wpack8[:b.shape[0], i * H:(i + 1) * H] = b
    blocks = [
        ww1[:, 2 * H:].T,                 # w1hT          [H, H]
        W_hh[:H].T,                       # WhhT_r        [H, H]
        W_hh[H:2 * H].T,                  # WhhT_z        [H, H]
        0.5 * W_hh[2 * H:].T,             # WhhT_nh       [H, H]
        np.eye(H, dtype=f),               # I128          [H, H]
        vv1[:, None],                     # vv1c          [H, 1]
        vv2[:, None],                     # vv2c          [H, 1]
    ]
    packw = sum(b.shape[1] for b in blocks)
    wpack = np.zeros((H, packw), f)
    o = 0
    for b in blocks:
        wpack[:, o:o + b.shape[1]] = b
        o += b.shape[1]
    shared = {"Wpack8": wpack8, "Wpack128": wpack}
    in_maps = []
    for c in range(NCORES):
        bs = slice(c * BL, (c + 1) * BL)
        pen = np.where(dynamic[bs, 0, :] != 0, NEG, 0.0).astype(f)
        pen[:, 0] = NEG
        m = dict(shared)
        m["sd12"] = ca(np.concatenate(
            [static[bs].transpose(1, 0, 2).reshape(SS, BL * S),
             dynamic[bs].transpose(1, 0, 2).reshape(DS, BL * S)]), f)
        m["penT0"] = ca(pen.T, f)
        in_maps.append(m)
    return in_maps


def unpack_outputs(results, n_steps=NSTEP):
    """results: list of 8 dicts with out_idx2/out_logp2 [S, NG*nchunk]."""
    nchunk = (GB * n_steps + S - 1) // S
    idxs, logps = [], []
    for res in results:
        iraw = res["out_idx2"]
        lraw = res["out_logp2"]
        idx = np.zeros((BL, n_steps), np.int32)
        lp = np.zeros((BL, n_steps), np.float32)
        for g in range(NG):
            iflat = iraw[:, g * nchunk:(g + 1) * nchunk].T.reshape(-1)
            lflat = lraw[:, g * nchunk:(g + 1) * nchunk].T.reshape(-1)
            idx[g * GB:(g + 1) * GB, :] = \
                iflat[:GB * n_steps].reshape(n_steps, GB).T.astype(np.int32)
            lp[g * GB:(g + 1) * GB, :] = \
                -np.log(lflat[:GB * n_steps].reshape(n_steps, GB).T)
        idxs.append(idx)
        logps.append(lp)
    return np.concatenate(idxs, 0), np.concatenate(logps, 0)


_CACHE = {}


def kernel(static, dynamic, transition_time, W_s, b_s, W_d, b_d, W_dec, b_dec,
           vv1, ww1, vv2, ww2, W_ih, W_hh, b_ih, b_hh):
    for bias in (b_s, b_d, b_dec, b_ih, b_hh):
        assert not np.any(np.asarray(bias)), "kernel assumes zero biases"
    from concourse.bass_utils import run_bass_kernel_spmd
    if "nc" not in _CACHE:
        _CACHE["nc"] = _build_nc()
    in_maps = host_inputs(np.asarray(static), np.asarray(dynamic),
                          np.asarray(W_s), np.asarray(W_d), np.asarray(W_dec),
                          np.asarray(vv1), np.asarray(ww1), np.asarray(vv2),
                          np.asarray(ww2), np.asarray(W_ih), np.asarray(W_hh))
    res = run_bass_kernel_spmd(_CACHE["nc"], in_maps,
                               core_ids=list(range(NCORES)))
    return unpack_outputs(res.results)
